# revision 11
# baseline (speedup 1.0000x reference)
"""Trainium2 Bass kernel for MixedGeometricCoefficientAttention.

Math note: in the reference, REVERSION * diag(SCALAR_TABLE) == ones, so
k_geo == k and (with ALPHA=0.5) k_mix == k.  The module collapses to standard
multi-head attention with head_dim=8, H=8 heads, and per-head 8x8
block-diagonal input/output projections (blocks per Cl(3,0) grade).

Sharding: 8 cores = (batch b in 0..3) x (head-group hg in 0..1, 4 heads each).
Each core computes, for its batch and its 4 heads, the full flash-style
attention and the partial output projection (sum over its 4 heads), returning
outT [8, 2048].  Host sums the two head-group partials per batch and adds the
output bias.

Device layout per core (all bf16 unless noted):
  xT      [9, 2048]   x[b]^T with a ones row (for bias folding)
  wq/wk   [9, 128]    projection lhsT; col 32h+i = W[:, i] (+bias row); rest 0
  wv      [9, 36]     per head 9 cols: 8 value dims + a ones column (denom)
  wo      [128, 8]    output proj lhsT; row 32h+j = Wo_h[j, :]
  esel    [4, 128]    fp32 selector: esel[h, 32h+j]=1 (j<9) for denominator
                      replication via matmul
  qk_sb   [128, 4096] head h on partitions 32h..32h+8; Q^T in cols 0:2048,
                      K^T in cols 2048:4096
  v_sb    [128, 576]  V' per ktile: cols kt*36 + 9h + (0..8), col 8 = ones
Flash loop per query tile qt (512 queries): 64 units (h, kt) -> S^T matmul
(per-head row strip), exp on ScalarE (groups of 3 units = [128,1536]), PV
matmul accumulating into oacc strips; then normalize by the ones-column
denominators and apply the output projection.
"""

from contextlib import ExitStack

import numpy as np
import ml_dtypes

import concourse.bacc as bacc
import concourse.bass as bass
import concourse.tile as tile
import concourse.mybir as mybir
import concourse.bass_utils as bass_utils

dt = mybir.dt
BF16 = ml_dtypes.bfloat16

B, T, MV = 4, 2048, 8
H = 8
HPC = 4          # heads per core
SCALE = MV ** -0.5
GRADES = [np.array([0]), np.array([1, 2, 3]), np.array([4, 5, 6]), np.array([7])]

QT = 512         # query tile
NQT = T // QT    # 4
KT = 128         # key tile
NKT = T // KT    # 16
GROUP = 3        # units per exp group

_NC_CACHE = {}


def build_nc(reps: int = 1, do_exp: bool = True, do_pv: bool = True,
             do_tail: bool = True):
    nc = bacc.Bacc("TRN2", target_bir_lowering=False)

    xT = nc.dram_tensor("xT", [9, T], dt.bfloat16, kind="ExternalInput")
    wq = nc.dram_tensor("wq", [9, 128], dt.bfloat16, kind="ExternalInput")
    wk = nc.dram_tensor("wk", [9, 128], dt.bfloat16, kind="ExternalInput")
    wv = nc.dram_tensor("wv", [9, 9 * HPC], dt.bfloat16, kind="ExternalInput")
    wo = nc.dram_tensor("wo", [128, 8], dt.bfloat16, kind="ExternalInput")
    esel = nc.dram_tensor("esel", [128, 128], dt.float32, kind="ExternalInput")
    out = nc.dram_tensor("out", [8, T], dt.float32, kind="ExternalOutput")

    with tile.TileContext(nc) as tc, ExitStack() as ctx:
        sing = ctx.enter_context(tc.tile_pool(name="sing", bufs=1))
        ptp = ctx.enter_context(tc.tile_pool(name="ptp", bufs=3))
        onp = ctx.enter_context(tc.tile_pool(name="onp", bufs=2))
        stp = ctx.enter_context(tc.tile_pool(name="stp", bufs=2, space="PSUM"))
        tailp = ctx.enter_context(tc.tile_pool(name="tailp", bufs=1, space="PSUM"))
        oaccp = ctx.enter_context(tc.tile_pool(name="oaccp", bufs=1, space="PSUM"))

        # --- load inputs ---
        xT_sb = sing.tile([9, T], dt.bfloat16, tag="xT_sb")
        wq_sb = sing.tile([9, 128], dt.bfloat16, tag="wq_sb")
        wk_sb = sing.tile([9, 128], dt.bfloat16, tag="wk_sb")
        wv_sb = sing.tile([9, 9 * HPC], dt.bfloat16, tag="wv_sb")
        wo_sb = sing.tile([128, 8], dt.bfloat16, tag="wo_sb")
        esel_sb = sing.tile([128, 128], dt.float32, tag="esel_sb")
        nc.sync.dma_start(xT_sb[:], xT[:])
        nc.sync.dma_start(wq_sb[:], wq[:])
        nc.sync.dma_start(wk_sb[:], wk[:])
        nc.sync.dma_start(wv_sb[:], wv[:])
        nc.sync.dma_start(wo_sb[:], wo[:])
        nc.sync.dma_start(esel_sb[:], esel[:])

        qk_sb = sing.tile([128, 2 * T], dt.bfloat16, tag="qk_sb")
        v_sb = sing.tile([128, NKT * 9 * HPC], dt.bfloat16, tag="v_sb")
        out_sb = sing.tile([8, T], dt.float32, tag="out_sb")
        # whole-qt P^T buffer: unit u at cols u*QT (decouples exp from PV so
        # cross-engine waits are few and coarse — sem wake latency here is
        # ~66us per blocking wait, so fine-grained ping-pong is catastrophic)
        pt_all = sing.tile([128, NKT * HPC * QT], dt.bfloat16, tag="pt_all")

        # Gap partitions (32h+9..32h+31) are never matmul-written; keep them at
        # 1.0 so the full-tile reciprocal in the tail stays finite.
        oacc = oaccp.tile([128, QT], dt.float32, tag="oacc")
        nc.vector.memset(oacc[:], 1.0)

        for _rep in range(reps):
            # --- Q/K projections: out[128, 512] per matmul ---
            for qk_i, w_sb in ((0, wq_sb), (1, wk_sb)):
                for half in range(2):
                    pj = stp.tile([128, 3 * QT], dt.float32, tag="st")
                    for j in range(2):
                        nc.tensor.matmul(
                            pj[:, j * QT:(j + 1) * QT],
                            w_sb[:],
                            xT_sb[:, (2 * half + j) * QT:(2 * half + j + 1) * QT],
                            start=True, stop=True,
                        )
                    nc.vector.tensor_copy(
                        qk_sb[:, qk_i * T + half * 1024: qk_i * T + (half + 1) * 1024],
                        pj[:, 0:1024],
                    )

            # --- V projection: [9,128].T @ [9,36] = [128,36] per ktile ---
            vps = stp.tile([128, 3 * QT], dt.float32, tag="st")
            W36 = 9 * HPC
            for kt in range(NKT):
                off = kt * W36 if kt < 8 else QT + (kt - 8) * W36
                nc.tensor.matmul(
                    vps[:, off:off + W36],
                    xT_sb[:, kt * KT:(kt + 1) * KT],
                    wv_sb[:],
                    start=True, stop=True,
                )
            nc.vector.tensor_copy(v_sb[:, 0:8 * W36], vps[:, 0:8 * W36])
            nc.vector.tensor_copy(v_sb[:, 8 * W36:16 * W36], vps[:, QT:QT + 8 * W36])

            # --- flash attention per query tile ---
            units = [(h, kt) for kt in range(NKT) for h in range(HPC)]
            for qt in range(NQT):
                qs = qt * QT
                # phase A: all S^T + exp for this qt, P^T staged to SBUF
                for g0 in range(0, len(units), GROUP):
                    grp = units[g0:g0 + GROUP]
                    st = stp.tile([128, 3 * QT], dt.float32, tag="st")
                    for i, (h, kt) in enumerate(grp):
                        nc.tensor.matmul(
                            st[:, i * QT:(i + 1) * QT],
                            qk_sb[32 * h:32 * h + 8, T + kt * KT:T + (kt + 1) * KT],
                            qk_sb[32 * h:32 * h + 8, qs:qs + QT],
                            start=True, stop=True,
                            tile_position=(32 * h, 0),
                        )
                    if not do_exp:
                        continue
                    n = len(grp)
                    nc.scalar.activation(
                        pt_all[:, g0 * QT:(g0 + n) * QT], st[:, 0:n * QT],
                        mybir.ActivationFunctionType.Exp,
                    )
                if not do_pv:
                    continue
                # phase B: all PV matmuls for this qt from the SBUF stage
                for u, (h, kt) in enumerate(units):
                    nc.tensor.matmul(
                        oacc[32 * h:32 * h + 9, :],
                        v_sb[:, kt * W36 + 9 * h:kt * W36 + 9 * h + 9],
                        pt_all[:, u * QT:(u + 1) * QT],
                        start=(kt == 0), stop=(kt == NKT - 1),
                        tile_position=(0, 32 * h),
                        skip_group_check=True,
                    )

                if not do_tail:
                    continue
                # --- normalize + output projection for this qt ---
                # reciprocal of the whole accumulator (only denominator rows
                # 32h+8 are meaningful; gaps are 1.0), then gather+replicate
                # the denominator reciprocals across each head's strip via the
                # esel selector matmul.
                os_sb = onp.tile([128, QT], dt.float32, tag="os_sb")
                nc.vector.tensor_copy(os_sb[:], oacc[:])
                rec = onp.tile([128, QT], dt.float32, tag="rec")
                nc.vector.reciprocal_approx_fast(rec[:], os_sb[:])

                rps = tailp.tile([128, QT], dt.float32, tag="tail")
                nc.tensor.matmul(rps[:], esel_sb[:], rec[:], start=True, stop=True)
                r_sb = onp.tile([128, QT], dt.float32, tag="r_sb")
                nc.vector.tensor_copy(r_sb[:], rps[:])

                onorm = onp.tile([128, QT], dt.bfloat16, tag="onorm")
                nc.vector.tensor_mul(onorm[:], oacc[:], r_sb[:])

                ops_ = tailp.tile([128, QT], dt.float32, tag="tail")
                nc.tensor.matmul(
                    ops_[0:8, :], wo_sb[:], onorm[:], start=True, stop=True
                )
                nc.vector.tensor_copy(out_sb[:, qs:qs + QT], ops_[0:8, :])

        nc.sync.dma_start(out[:], out_sb[:])

    nc.compile()
    return nc


# ---------------------------------------------------------------------------
# host side
# ---------------------------------------------------------------------------

def _full_mats(ws, bs, transpose_in=False):
    """Per-head full 8x8 matrices + bias from per-grade packed params."""
    W = np.zeros((H, 8, 8), np.float32)
    bias = np.zeros((H, 8), np.float32)
    for idx, w, b in zip(GRADES, ws, bs):
        d = len(idx)
        w = np.asarray(w, np.float32)
        b = np.asarray(b, np.float32)
        for h in range(H):
            if transpose_in:
                W[h][np.ix_(idx, idx)] = w[h * d:(h + 1) * d, :]
            else:
                W[h][np.ix_(idx, idx)] = w[:, h * d:(h + 1) * d]
            if not transpose_in:
                bias[h][idx] = b[h * d:(h + 1) * d]
    return W, bias


def _host_inputs(inputs):
    x = np.asarray(inputs["x"], np.float32)
    Wq, bq = _full_mats(inputs["q_ws"], inputs["q_bs"])
    Wk, bk = _full_mats(inputs["k_ws"], inputs["k_bs"])
    Wv, bv = _full_mats(inputs["v_ws"], inputs["v_bs"])
    Wo, _ = _full_mats(inputs["o_ws"], inputs["o_bs"], transpose_in=True)
    bo = np.zeros(8, np.float32)
    for idx, b in zip(GRADES, inputs["o_bs"]):
        bo[idx] = np.asarray(b, np.float32)

    in_maps = []
    for c in range(8):
        b, hg = c // 2, c % 2
        heads = [hg * HPC + h for h in range(HPC)]

        xT9 = np.zeros((9, T), np.float32)
        xT9[0:8] = x[b].T
        xT9[8] = 1.0

        wq128 = np.zeros((9, 128), np.float32)
        wk128 = np.zeros((9, 128), np.float32)
        for h, hh in enumerate(heads):
            wq128[0:8, 32 * h:32 * h + 8] = Wq[hh] * SCALE
            wq128[8, 32 * h:32 * h + 8] = bq[hh] * SCALE
            wk128[0:8, 32 * h:32 * h + 8] = Wk[hh]
            wk128[8, 32 * h:32 * h + 8] = bk[hh]

        wv36 = np.zeros((9, 9 * HPC), np.float32)
        for h, hh in enumerate(heads):
            wv36[0:8, 9 * h:9 * h + 8] = Wv[hh]
            wv36[8, 9 * h:9 * h + 8] = bv[hh]
            wv36[8, 9 * h + 8] = 1.0

        wo128 = np.zeros((128, 8), np.float32)
        for h, hh in enumerate(heads):
            wo128[32 * h:32 * h + 8, :] = Wo[hh]

        # esel[p, m] = 1 iff p == 32h+8 and m in [32h, 32h+9): the replicate
        # matmul R = esel^T-contract(rec) puts 1/den_h on each strip's rows.
        es = np.zeros((128, 128), np.float32)
        for h in range(HPC):
            es[32 * h + 8, 32 * h:32 * h + 9] = 1.0

        in_maps.append({
            "xT": xT9.astype(BF16),
            "wq": wq128.astype(BF16),
            "wk": wk128.astype(BF16),
            "wv": wv36.astype(BF16),
            "wo": wo128.astype(BF16),
            "esel": es,
        })
    return in_maps, bo


def _numpy_fallback(inputs):
    """Exact-math fallback (used only if a nonzero padding mask appears)."""
    x = np.asarray(inputs["x"], np.float32)
    mask = np.asarray(inputs["key_padding_mask"])
    Wq, bq = _full_mats(inputs["q_ws"], inputs["q_bs"])
    Wk, bk = _full_mats(inputs["k_ws"], inputs["k_bs"])
    Wv, bv = _full_mats(inputs["v_ws"], inputs["v_bs"])
    Wo, _ = _full_mats(inputs["o_ws"], inputs["o_bs"], transpose_in=True)
    bo = np.zeros(8, np.float32)
    for idx, b in zip(GRADES, inputs["o_bs"]):
        bo[idx] = np.asarray(b, np.float32)
    out = np.zeros_like(x)
    for b in range(B):
        for h in range(H):
            q = x[b] @ (Wq[h] * SCALE) + bq[h] * SCALE
            k = x[b] @ Wk[h] + bk[h]
            v = x[b] @ Wv[h] + bv[h]
            s = q @ k.T
            s = np.where(mask[b][None, :], -np.inf, s)
            s = s - s.max(axis=1, keepdims=True)
            p = np.exp(s)
            p /= p.sum(axis=1, keepdims=True)
            out[b] += (p @ v) @ Wo[h]
        out[b] += bo
    return out


def kernel(**inputs) -> np.ndarray:
    mask = np.asarray(inputs["key_padding_mask"])
    if mask.any():
        return _numpy_fallback(inputs)

    in_maps, bo = _host_inputs(inputs)
    if "nc" not in _NC_CACHE:
        _NC_CACHE["nc"] = build_nc()
    nc = _NC_CACHE["nc"]
    res = bass_utils.run_bass_kernel_spmd(nc, in_maps, core_ids=list(range(8)))
    out = np.zeros((B, T, MV), np.float32)
    for b in range(B):
        acc = res.results[2 * b]["out"] + res.results[2 * b + 1]["out"]
        out[b] = acc.T + bo
    return out


# revision 14
# speedup vs baseline: 1.2809x; 1.2809x over previous
"""Trainium2 Bass kernel for MixedGeometricCoefficientAttention.

Math note: in the reference, REVERSION * diag(SCALAR_TABLE) == ones, so
k_geo == k and (with ALPHA=0.5) k_mix == k.  The module collapses to standard
multi-head attention with head_dim=8, H=8 heads, and per-head 8x8
block-diagonal input/output projections (blocks per Cl(3,0) grade).

Sharding: 8 cores = (batch b in 0..3) x (head-group hg in 0..1, 4 heads each).
Each core computes, for its batch and its 4 heads, the full flash-style
attention and the partial output projection (sum over its 4 heads), returning
outT [8, 2048].  Host sums the two head-group partials per batch and adds the
output bias.

Device layout per core (all bf16 unless noted):
  xT      [9, 2048]   x[b]^T with a ones row (for bias folding)
  wq/wk   [9, 128]    projection lhsT; col 32h+i = W[:, i] (+bias row); rest 0
  wv      [9, 36]     per head 9 cols: 8 value dims + a ones column (denom)
  wo      [128, 8]    output proj lhsT; row 32h+j = Wo_h[j, :]
  esel    [4, 128]    fp32 selector: esel[h, 32h+j]=1 (j<9) for denominator
                      replication via matmul
  qk_sb   [128, 4096] head h on partitions 32h..32h+8; Q^T in cols 0:2048,
                      K^T in cols 2048:4096
  v_sb    [128, 576]  V' per ktile: cols kt*36 + 9h + (0..8), col 8 = ones
Flash loop per query tile qt (512 queries): 64 units (h, kt) -> S^T matmul
(per-head row strip), exp on ScalarE (groups of 3 units = [128,1536]), PV
matmul accumulating into oacc strips; then normalize by the ones-column
denominators and apply the output projection.
"""

from contextlib import ExitStack

import numpy as np
import ml_dtypes

import concourse.bacc as bacc
import concourse.bass as bass
import concourse.tile as tile
import concourse.mybir as mybir
import concourse.bass_utils as bass_utils

dt = mybir.dt
BF16 = ml_dtypes.bfloat16

B, T, MV = 4, 2048, 8
H = 8
HPC = 4          # heads per core
SCALE = MV ** -0.5
GRADES = [np.array([0]), np.array([1, 2, 3]), np.array([4, 5, 6]), np.array([7])]

QT = 512         # query tile
NQT = T // QT    # 4
KT = 128         # key tile
NKT = T // KT    # 16
GROUP = 3        # units per exp group

_NC_CACHE = {}


def build_nc(reps: int = 1, do_exp: bool = True, do_pv: bool = True,
             do_tail: bool = True, staged: bool = False):
    nc = bacc.Bacc("TRN2", target_bir_lowering=False)

    xT = nc.dram_tensor("xT", [9, T], dt.bfloat16, kind="ExternalInput")
    wq = nc.dram_tensor("wq", [9, 128], dt.bfloat16, kind="ExternalInput")
    wk = nc.dram_tensor("wk", [9, 128], dt.bfloat16, kind="ExternalInput")
    wv = nc.dram_tensor("wv", [9, 9 * HPC], dt.bfloat16, kind="ExternalInput")
    wo = nc.dram_tensor("wo", [128, 8], dt.bfloat16, kind="ExternalInput")
    esel = nc.dram_tensor("esel", [128, 128], dt.float32, kind="ExternalInput")
    out = nc.dram_tensor("out", [8, T], dt.float32, kind="ExternalOutput")

    with tile.TileContext(nc) as tc, ExitStack() as ctx:
        sing = ctx.enter_context(tc.tile_pool(name="sing", bufs=1))
        ptp = ctx.enter_context(tc.tile_pool(name="ptp", bufs=3))
        onp = ctx.enter_context(tc.tile_pool(name="onp", bufs=2))
        stp = ctx.enter_context(tc.tile_pool(name="stp", bufs=2, space="PSUM"))
        tailp = ctx.enter_context(tc.tile_pool(name="tailp", bufs=1, space="PSUM"))
        oaccp = ctx.enter_context(tc.tile_pool(name="oaccp", bufs=1, space="PSUM"))

        # --- load inputs ---
        xT_sb = sing.tile([9, T], dt.bfloat16, tag="xT_sb")
        wq_sb = sing.tile([9, 128], dt.bfloat16, tag="wq_sb")
        wk_sb = sing.tile([9, 128], dt.bfloat16, tag="wk_sb")
        wv_sb = sing.tile([9, 9 * HPC], dt.bfloat16, tag="wv_sb")
        wo_sb = sing.tile([128, 8], dt.bfloat16, tag="wo_sb")
        esel_sb = sing.tile([128, 128], dt.float32, tag="esel_sb")
        nc.sync.dma_start(xT_sb[:], xT[:])
        nc.sync.dma_start(wq_sb[:], wq[:])
        nc.sync.dma_start(wk_sb[:], wk[:])
        nc.sync.dma_start(wv_sb[:], wv[:])
        nc.sync.dma_start(wo_sb[:], wo[:])
        nc.sync.dma_start(esel_sb[:], esel[:])

        qk_sb = sing.tile([128, 2 * T], dt.bfloat16, tag="qk_sb")
        v_sb = sing.tile([128, NKT * 9 * HPC], dt.bfloat16, tag="v_sb")
        out_sb = sing.tile([8, T], dt.float32, tag="out_sb")
        # whole-qt P^T buffer: unit u at cols u*QT (decouples exp from PV so
        # cross-engine waits are few and coarse — sem wake latency here is
        # ~66us per blocking wait, so fine-grained ping-pong is catastrophic)
        pt_all = sing.tile([128, NKT * HPC * QT], dt.bfloat16, tag="pt_all")

        # Gap partitions (32h+9..32h+31) are never matmul-written; keep them at
        # 1.0 so the full-tile reciprocal in the tail stays finite.
        oacc = oaccp.tile([128, QT], dt.float32, tag="oacc")
        nc.vector.memset(oacc[:], 1.0)

        for _rep in range(reps):
            # --- Q/K projections: out[128, 512] per matmul ---
            for qk_i, w_sb in ((0, wq_sb), (1, wk_sb)):
                for half in range(2):
                    pj = stp.tile([128, 3 * QT], dt.float32, tag="st")
                    for j in range(2):
                        nc.tensor.matmul(
                            pj[:, j * QT:(j + 1) * QT],
                            w_sb[:],
                            xT_sb[:, (2 * half + j) * QT:(2 * half + j + 1) * QT],
                            start=True, stop=True,
                        )
                    nc.vector.tensor_copy(
                        qk_sb[:, qk_i * T + half * 1024: qk_i * T + (half + 1) * 1024],
                        pj[:, 0:1024],
                    )

            # --- V projection: [9,128].T @ [9,36] = [128,36] per ktile ---
            vps = stp.tile([128, 3 * QT], dt.float32, tag="st")
            W36 = 9 * HPC
            for kt in range(NKT):
                off = kt * W36 if kt < 8 else QT + (kt - 8) * W36
                nc.tensor.matmul(
                    vps[:, off:off + W36],
                    xT_sb[:, kt * KT:(kt + 1) * KT],
                    wv_sb[:],
                    start=True, stop=True,
                )
            nc.vector.tensor_copy(v_sb[:, 0:8 * W36], vps[:, 0:8 * W36])
            nc.vector.tensor_copy(v_sb[:, 8 * W36:16 * W36], vps[:, QT:QT + 8 * W36])

            # --- flash attention per query tile ---
            units = [(h, kt) for kt in range(NKT) for h in range(HPC)]
            for qt in range(NQT):
                qs = qt * QT
                # phase A: all S^T + exp for this qt, P^T staged to SBUF
                for g0 in range(0, len(units), GROUP):
                    grp = units[g0:g0 + GROUP]
                    st = stp.tile([128, 3 * QT], dt.float32, tag="st")
                    for i, (h, kt) in enumerate(grp):
                        nc.tensor.matmul(
                            st[:, i * QT:(i + 1) * QT],
                            qk_sb[32 * h:32 * h + 8, T + kt * KT:T + (kt + 1) * KT],
                            qk_sb[32 * h:32 * h + 8, qs:qs + QT],
                            start=True, stop=True,
                            tile_position=(32 * h, 0),
                        )
                    if not do_exp:
                        continue
                    n = len(grp)
                    nc.scalar.activation(
                        pt_all[:, g0 * QT:(g0 + n) * QT], st[:, 0:n * QT],
                        mybir.ActivationFunctionType.Exp,
                    )
                    if do_pv and not staged:
                        for i, (h, kt) in enumerate(grp):
                            nc.tensor.matmul(
                                oacc[32 * h:32 * h + 9, :],
                                v_sb[:, kt * W36 + 9 * h:kt * W36 + 9 * h + 9],
                                pt_all[:, (g0 + i) * QT:(g0 + i + 1) * QT],
                                start=(kt == 0), stop=(kt == NKT - 1),
                                tile_position=(0, 32 * h),
                                skip_group_check=True,
                            )
                if not do_pv:
                    continue
                if staged:
                    # phase B: all PV matmuls for this qt from the SBUF stage
                    for u, (h, kt) in enumerate(units):
                        nc.tensor.matmul(
                            oacc[32 * h:32 * h + 9, :],
                            v_sb[:, kt * W36 + 9 * h:kt * W36 + 9 * h + 9],
                            pt_all[:, u * QT:(u + 1) * QT],
                            start=(kt == 0), stop=(kt == NKT - 1),
                            tile_position=(0, 32 * h),
                            skip_group_check=True,
                        )

                if not do_tail:
                    continue
                # --- normalize + output projection for this qt ---
                # reciprocal of the whole accumulator (only denominator rows
                # 32h+8 are meaningful; gaps are 1.0), then gather+replicate
                # the denominator reciprocals across each head's strip via the
                # esel selector matmul.
                os_sb = onp.tile([128, QT], dt.float32, tag="os_sb")
                nc.vector.tensor_copy(os_sb[:], oacc[:])
                rec = onp.tile([128, QT], dt.float32, tag="rec")
                nc.vector.reciprocal_approx_fast(rec[:], os_sb[:])

                rps = tailp.tile([128, QT], dt.float32, tag="tail")
                nc.tensor.matmul(rps[:], esel_sb[:], rec[:], start=True, stop=True)
                r_sb = onp.tile([128, QT], dt.float32, tag="r_sb")
                nc.vector.tensor_copy(r_sb[:], rps[:])

                onorm = onp.tile([128, QT], dt.bfloat16, tag="onorm")
                nc.vector.tensor_mul(onorm[:], oacc[:], r_sb[:])

                ops_ = tailp.tile([128, QT], dt.float32, tag="tail")
                nc.tensor.matmul(
                    ops_[0:8, :], wo_sb[:], onorm[:], start=True, stop=True
                )
                nc.vector.tensor_copy(out_sb[:, qs:qs + QT], ops_[0:8, :])

        nc.sync.dma_start(out[:], out_sb[:])

    nc.compile()
    return nc


# ---------------------------------------------------------------------------
# host side
# ---------------------------------------------------------------------------

def _full_mats(ws, bs, transpose_in=False):
    """Per-head full 8x8 matrices + bias from per-grade packed params."""
    W = np.zeros((H, 8, 8), np.float32)
    bias = np.zeros((H, 8), np.float32)
    for idx, w, b in zip(GRADES, ws, bs):
        d = len(idx)
        w = np.asarray(w, np.float32)
        b = np.asarray(b, np.float32)
        for h in range(H):
            if transpose_in:
                W[h][np.ix_(idx, idx)] = w[h * d:(h + 1) * d, :]
            else:
                W[h][np.ix_(idx, idx)] = w[:, h * d:(h + 1) * d]
            if not transpose_in:
                bias[h][idx] = b[h * d:(h + 1) * d]
    return W, bias


def _host_inputs(inputs):
    x = np.asarray(inputs["x"], np.float32)
    Wq, bq = _full_mats(inputs["q_ws"], inputs["q_bs"])
    Wk, bk = _full_mats(inputs["k_ws"], inputs["k_bs"])
    Wv, bv = _full_mats(inputs["v_ws"], inputs["v_bs"])
    Wo, _ = _full_mats(inputs["o_ws"], inputs["o_bs"], transpose_in=True)
    bo = np.zeros(8, np.float32)
    for idx, b in zip(GRADES, inputs["o_bs"]):
        bo[idx] = np.asarray(b, np.float32)

    in_maps = []
    for c in range(8):
        b, hg = c // 2, c % 2
        heads = [hg * HPC + h for h in range(HPC)]

        xT9 = np.zeros((9, T), np.float32)
        xT9[0:8] = x[b].T
        xT9[8] = 1.0

        wq128 = np.zeros((9, 128), np.float32)
        wk128 = np.zeros((9, 128), np.float32)
        for h, hh in enumerate(heads):
            wq128[0:8, 32 * h:32 * h + 8] = Wq[hh] * SCALE
            wq128[8, 32 * h:32 * h + 8] = bq[hh] * SCALE
            wk128[0:8, 32 * h:32 * h + 8] = Wk[hh]
            wk128[8, 32 * h:32 * h + 8] = bk[hh]

        wv36 = np.zeros((9, 9 * HPC), np.float32)
        for h, hh in enumerate(heads):
            wv36[0:8, 9 * h:9 * h + 8] = Wv[hh]
            wv36[8, 9 * h:9 * h + 8] = bv[hh]
            wv36[8, 9 * h + 8] = 1.0

        wo128 = np.zeros((128, 8), np.float32)
        for h, hh in enumerate(heads):
            wo128[32 * h:32 * h + 8, :] = Wo[hh]

        # esel[p, m] = 1 iff p == 32h+8 and m in [32h, 32h+9): the replicate
        # matmul R = esel^T-contract(rec) puts 1/den_h on each strip's rows.
        es = np.zeros((128, 128), np.float32)
        for h in range(HPC):
            es[32 * h + 8, 32 * h:32 * h + 9] = 1.0

        in_maps.append({
            "xT": xT9.astype(BF16),
            "wq": wq128.astype(BF16),
            "wk": wk128.astype(BF16),
            "wv": wv36.astype(BF16),
            "wo": wo128.astype(BF16),
            "esel": es,
        })
    return in_maps, bo


def _numpy_fallback(inputs):
    """Exact-math fallback (used only if a nonzero padding mask appears)."""
    x = np.asarray(inputs["x"], np.float32)
    mask = np.asarray(inputs["key_padding_mask"])
    Wq, bq = _full_mats(inputs["q_ws"], inputs["q_bs"])
    Wk, bk = _full_mats(inputs["k_ws"], inputs["k_bs"])
    Wv, bv = _full_mats(inputs["v_ws"], inputs["v_bs"])
    Wo, _ = _full_mats(inputs["o_ws"], inputs["o_bs"], transpose_in=True)
    bo = np.zeros(8, np.float32)
    for idx, b in zip(GRADES, inputs["o_bs"]):
        bo[idx] = np.asarray(b, np.float32)
    out = np.zeros_like(x)
    for b in range(B):
        for h in range(H):
            q = x[b] @ (Wq[h] * SCALE) + bq[h] * SCALE
            k = x[b] @ Wk[h] + bk[h]
            v = x[b] @ Wv[h] + bv[h]
            s = q @ k.T
            s = np.where(mask[b][None, :], -np.inf, s)
            s = s - s.max(axis=1, keepdims=True)
            p = np.exp(s)
            p /= p.sum(axis=1, keepdims=True)
            out[b] += (p @ v) @ Wo[h]
        out[b] += bo
    return out


def kernel(**inputs) -> np.ndarray:
    mask = np.asarray(inputs["key_padding_mask"])
    if mask.any():
        return _numpy_fallback(inputs)

    in_maps, bo = _host_inputs(inputs)
    if "nc" not in _NC_CACHE:
        _NC_CACHE["nc"] = build_nc()
    nc = _NC_CACHE["nc"]
    res = bass_utils.run_bass_kernel_spmd(nc, in_maps, core_ids=list(range(8)))
    out = np.zeros((B, T, MV), np.float32)
    for b in range(B):
        acc = res.results[2 * b]["out"] + res.results[2 * b + 1]["out"]
        out[b] = acc.T + bo
    return out


# revision 15
# speedup vs baseline: 1.4648x; 1.1436x over previous
"""Trainium2 Bass kernel for MixedGeometricCoefficientAttention.

Math note: in the reference, REVERSION * diag(SCALAR_TABLE) == ones, so
k_geo == k and (with ALPHA=0.5) k_mix == k.  The module collapses to standard
multi-head attention with head_dim=8, H=8 heads, and per-head 8x8
block-diagonal input/output projections (blocks per Cl(3,0) grade).

Sharding: 8 cores = (batch b in 0..3) x (head-group hg in 0..1, 4 heads each).
Each core computes, for its batch and its 4 heads, the full flash-style
attention and the partial output projection (sum over its 4 heads), returning
outT [8, 2048].  Host sums the two head-group partials per batch and adds the
output bias.

Device layout per core (all bf16 unless noted):
  xT      [9, 2048]   x[b]^T with a ones row (for bias folding)
  wq/wk   [9, 128]    projection lhsT; col 32h+i = W[:, i] (+bias row); rest 0
  wv      [9, 36]     per head 9 cols: 8 value dims + a ones column (denom)
  wo      [128, 8]    output proj lhsT; row 32h+j = Wo_h[j, :]
  esel    [4, 128]    fp32 selector: esel[h, 32h+j]=1 (j<9) for denominator
                      replication via matmul
  qk_sb   [128, 4096] head h on partitions 32h..32h+8; Q^T in cols 0:2048,
                      K^T in cols 2048:4096
  v_sb    [128, 576]  V' per ktile: cols kt*36 + 9h + (0..8), col 8 = ones
Flash loop per query tile qt (512 queries): 64 units (h, kt) -> S^T matmul
(per-head row strip), exp on ScalarE (groups of 3 units = [128,1536]), PV
matmul accumulating into oacc strips; then normalize by the ones-column
denominators and apply the output projection.
"""

from contextlib import ExitStack

import numpy as np
import ml_dtypes

import concourse.bacc as bacc
import concourse.bass as bass
import concourse.tile as tile
import concourse.mybir as mybir
import concourse.bass_utils as bass_utils

dt = mybir.dt
BF16 = ml_dtypes.bfloat16

B, T, MV = 4, 2048, 8
H = 8
HPC = 4          # heads per core
SCALE = MV ** -0.5
GRADES = [np.array([0]), np.array([1, 2, 3]), np.array([4, 5, 6]), np.array([7])]

QT = 512         # query tile
NQT = T // QT    # 4
KT = 128         # key tile
NKT = T // KT    # 16
GROUP = 6        # units per exp group (single-buffered 6-bank S^T tile:
                 # fewer, coarser cross-engine handoffs — sync latency on
                 # this environment dwarfs engine-overlap losses)

_NC_CACHE = {}


def build_nc(reps: int = 1, do_exp: bool = True, do_pv: bool = True,
             do_tail: bool = True, staged: bool = False):
    nc = bacc.Bacc("TRN2", target_bir_lowering=False)

    xT = nc.dram_tensor("xT", [9, T], dt.bfloat16, kind="ExternalInput")
    wq = nc.dram_tensor("wq", [9, 128], dt.bfloat16, kind="ExternalInput")
    wk = nc.dram_tensor("wk", [9, 128], dt.bfloat16, kind="ExternalInput")
    wv = nc.dram_tensor("wv", [9, 9 * HPC], dt.bfloat16, kind="ExternalInput")
    wo = nc.dram_tensor("wo", [128, 8], dt.bfloat16, kind="ExternalInput")
    esel = nc.dram_tensor("esel", [128, 128], dt.float32, kind="ExternalInput")
    out = nc.dram_tensor("out", [8, T], dt.float32, kind="ExternalOutput")

    with tile.TileContext(nc) as tc, ExitStack() as ctx:
        sing = ctx.enter_context(tc.tile_pool(name="sing", bufs=1))
        ptp = ctx.enter_context(tc.tile_pool(name="ptp", bufs=3))
        onp = ctx.enter_context(tc.tile_pool(name="onp", bufs=2))
        stp = ctx.enter_context(tc.tile_pool(name="stp", bufs=1, space="PSUM"))
        tailp = ctx.enter_context(tc.tile_pool(name="tailp", bufs=1, space="PSUM"))
        oaccp = ctx.enter_context(tc.tile_pool(name="oaccp", bufs=1, space="PSUM"))

        # --- load inputs ---
        xT_sb = sing.tile([9, T], dt.bfloat16, tag="xT_sb")
        wq_sb = sing.tile([9, 128], dt.bfloat16, tag="wq_sb")
        wk_sb = sing.tile([9, 128], dt.bfloat16, tag="wk_sb")
        wv_sb = sing.tile([9, 9 * HPC], dt.bfloat16, tag="wv_sb")
        wo_sb = sing.tile([128, 8], dt.bfloat16, tag="wo_sb")
        esel_sb = sing.tile([128, 128], dt.float32, tag="esel_sb")
        nc.sync.dma_start(xT_sb[:], xT[:])
        nc.sync.dma_start(wq_sb[:], wq[:])
        nc.sync.dma_start(wk_sb[:], wk[:])
        nc.sync.dma_start(wv_sb[:], wv[:])
        nc.sync.dma_start(wo_sb[:], wo[:])
        nc.sync.dma_start(esel_sb[:], esel[:])

        qk_sb = sing.tile([128, 2 * T], dt.bfloat16, tag="qk_sb")
        v_sb = sing.tile([128, NKT * 9 * HPC], dt.bfloat16, tag="v_sb")
        out_sb = sing.tile([8, T], dt.float32, tag="out_sb")
        # whole-qt P^T buffer: unit u at cols u*QT (decouples exp from PV so
        # cross-engine waits are few and coarse — sem wake latency here is
        # ~66us per blocking wait, so fine-grained ping-pong is catastrophic)
        pt_all = sing.tile([128, NKT * HPC * QT], dt.bfloat16, tag="pt_all")

        # Gap partitions (32h+9..32h+31) are never matmul-written; keep them at
        # 1.0 so the full-tile reciprocal in the tail stays finite.
        oacc = oaccp.tile([128, QT], dt.float32, tag="oacc")
        nc.vector.memset(oacc[:], 1.0)

        for _rep in range(reps):
            # --- Q/K projections: out[128, 512] per matmul ---
            for qk_i, w_sb in ((0, wq_sb), (1, wk_sb)):
                for half in range(2):
                    pj = stp.tile([128, GROUP * QT], dt.float32, tag="st")
                    for j in range(2):
                        nc.tensor.matmul(
                            pj[:, j * QT:(j + 1) * QT],
                            w_sb[:],
                            xT_sb[:, (2 * half + j) * QT:(2 * half + j + 1) * QT],
                            start=True, stop=True,
                        )
                    nc.vector.tensor_copy(
                        qk_sb[:, qk_i * T + half * 1024: qk_i * T + (half + 1) * 1024],
                        pj[:, 0:1024],
                    )

            # --- V projection: [9,128].T @ [9,36] = [128,36] per ktile ---
            vps = stp.tile([128, GROUP * QT], dt.float32, tag="st")
            W36 = 9 * HPC
            for kt in range(NKT):
                off = kt * W36 if kt < 8 else QT + (kt - 8) * W36
                nc.tensor.matmul(
                    vps[:, off:off + W36],
                    xT_sb[:, kt * KT:(kt + 1) * KT],
                    wv_sb[:],
                    start=True, stop=True,
                )
            nc.vector.tensor_copy(v_sb[:, 0:8 * W36], vps[:, 0:8 * W36])
            nc.vector.tensor_copy(v_sb[:, 8 * W36:16 * W36], vps[:, QT:QT + 8 * W36])

            # --- flash attention per query tile ---
            units = [(h, kt) for kt in range(NKT) for h in range(HPC)]
            for qt in range(NQT):
                qs = qt * QT
                # phase A: all S^T + exp for this qt, P^T staged to SBUF
                for g0 in range(0, len(units), GROUP):
                    grp = units[g0:g0 + GROUP]
                    st = stp.tile([128, GROUP * QT], dt.float32, tag="st")
                    for i, (h, kt) in enumerate(grp):
                        nc.tensor.matmul(
                            st[:, i * QT:(i + 1) * QT],
                            qk_sb[32 * h:32 * h + 8, T + kt * KT:T + (kt + 1) * KT],
                            qk_sb[32 * h:32 * h + 8, qs:qs + QT],
                            start=True, stop=True,
                            tile_position=(32 * h, 0),
                        )
                    if not do_exp:
                        continue
                    n = len(grp)
                    nc.scalar.activation(
                        pt_all[:, g0 * QT:(g0 + n) * QT], st[:, 0:n * QT],
                        mybir.ActivationFunctionType.Exp,
                    )
                    if do_pv and not staged:
                        for i, (h, kt) in enumerate(grp):
                            nc.tensor.matmul(
                                oacc[32 * h:32 * h + 9, :],
                                v_sb[:, kt * W36 + 9 * h:kt * W36 + 9 * h + 9],
                                pt_all[:, (g0 + i) * QT:(g0 + i + 1) * QT],
                                start=(kt == 0), stop=(kt == NKT - 1),
                                tile_position=(0, 32 * h),
                                skip_group_check=True,
                            )
                if not do_pv:
                    continue
                if staged:
                    # phase B: all PV matmuls for this qt from the SBUF stage
                    for u, (h, kt) in enumerate(units):
                        nc.tensor.matmul(
                            oacc[32 * h:32 * h + 9, :],
                            v_sb[:, kt * W36 + 9 * h:kt * W36 + 9 * h + 9],
                            pt_all[:, u * QT:(u + 1) * QT],
                            start=(kt == 0), stop=(kt == NKT - 1),
                            tile_position=(0, 32 * h),
                            skip_group_check=True,
                        )

                if not do_tail:
                    continue
                # --- normalize + output projection for this qt ---
                # reciprocal of the whole accumulator (only denominator rows
                # 32h+8 are meaningful; gaps are 1.0), then gather+replicate
                # the denominator reciprocals across each head's strip via the
                # esel selector matmul.
                os_sb = onp.tile([128, QT], dt.float32, tag="os_sb")
                nc.vector.tensor_copy(os_sb[:], oacc[:])
                rec = onp.tile([128, QT], dt.float32, tag="rec")
                nc.vector.reciprocal_approx_fast(rec[:], os_sb[:])

                rps = tailp.tile([128, QT], dt.float32, tag="tail")
                nc.tensor.matmul(rps[:], esel_sb[:], rec[:], start=True, stop=True)
                r_sb = onp.tile([128, QT], dt.float32, tag="r_sb")
                nc.vector.tensor_copy(r_sb[:], rps[:])

                onorm = onp.tile([128, QT], dt.bfloat16, tag="onorm")
                nc.vector.tensor_mul(onorm[:], oacc[:], r_sb[:])

                ops_ = tailp.tile([128, QT], dt.float32, tag="tail")
                nc.tensor.matmul(
                    ops_[0:8, :], wo_sb[:], onorm[:], start=True, stop=True
                )
                nc.vector.tensor_copy(out_sb[:, qs:qs + QT], ops_[0:8, :])

        nc.sync.dma_start(out[:], out_sb[:])

    nc.compile()
    return nc


# ---------------------------------------------------------------------------
# host side
# ---------------------------------------------------------------------------

def _full_mats(ws, bs, transpose_in=False):
    """Per-head full 8x8 matrices + bias from per-grade packed params."""
    W = np.zeros((H, 8, 8), np.float32)
    bias = np.zeros((H, 8), np.float32)
    for idx, w, b in zip(GRADES, ws, bs):
        d = len(idx)
        w = np.asarray(w, np.float32)
        b = np.asarray(b, np.float32)
        for h in range(H):
            if transpose_in:
                W[h][np.ix_(idx, idx)] = w[h * d:(h + 1) * d, :]
            else:
                W[h][np.ix_(idx, idx)] = w[:, h * d:(h + 1) * d]
            if not transpose_in:
                bias[h][idx] = b[h * d:(h + 1) * d]
    return W, bias


def _host_inputs(inputs):
    x = np.asarray(inputs["x"], np.float32)
    Wq, bq = _full_mats(inputs["q_ws"], inputs["q_bs"])
    Wk, bk = _full_mats(inputs["k_ws"], inputs["k_bs"])
    Wv, bv = _full_mats(inputs["v_ws"], inputs["v_bs"])
    Wo, _ = _full_mats(inputs["o_ws"], inputs["o_bs"], transpose_in=True)
    bo = np.zeros(8, np.float32)
    for idx, b in zip(GRADES, inputs["o_bs"]):
        bo[idx] = np.asarray(b, np.float32)

    in_maps = []
    for c in range(8):
        b, hg = c // 2, c % 2
        heads = [hg * HPC + h for h in range(HPC)]

        xT9 = np.zeros((9, T), np.float32)
        xT9[0:8] = x[b].T
        xT9[8] = 1.0

        wq128 = np.zeros((9, 128), np.float32)
        wk128 = np.zeros((9, 128), np.float32)
        for h, hh in enumerate(heads):
            wq128[0:8, 32 * h:32 * h + 8] = Wq[hh] * SCALE
            wq128[8, 32 * h:32 * h + 8] = bq[hh] * SCALE
            wk128[0:8, 32 * h:32 * h + 8] = Wk[hh]
            wk128[8, 32 * h:32 * h + 8] = bk[hh]

        wv36 = np.zeros((9, 9 * HPC), np.float32)
        for h, hh in enumerate(heads):
            wv36[0:8, 9 * h:9 * h + 8] = Wv[hh]
            wv36[8, 9 * h:9 * h + 8] = bv[hh]
            wv36[8, 9 * h + 8] = 1.0

        wo128 = np.zeros((128, 8), np.float32)
        for h, hh in enumerate(heads):
            wo128[32 * h:32 * h + 8, :] = Wo[hh]

        # esel[p, m] = 1 iff p == 32h+8 and m in [32h, 32h+9): the replicate
        # matmul R = esel^T-contract(rec) puts 1/den_h on each strip's rows.
        es = np.zeros((128, 128), np.float32)
        for h in range(HPC):
            es[32 * h + 8, 32 * h:32 * h + 9] = 1.0

        in_maps.append({
            "xT": xT9.astype(BF16),
            "wq": wq128.astype(BF16),
            "wk": wk128.astype(BF16),
            "wv": wv36.astype(BF16),
            "wo": wo128.astype(BF16),
            "esel": es,
        })
    return in_maps, bo


def _numpy_fallback(inputs):
    """Exact-math fallback (used only if a nonzero padding mask appears)."""
    x = np.asarray(inputs["x"], np.float32)
    mask = np.asarray(inputs["key_padding_mask"])
    Wq, bq = _full_mats(inputs["q_ws"], inputs["q_bs"])
    Wk, bk = _full_mats(inputs["k_ws"], inputs["k_bs"])
    Wv, bv = _full_mats(inputs["v_ws"], inputs["v_bs"])
    Wo, _ = _full_mats(inputs["o_ws"], inputs["o_bs"], transpose_in=True)
    bo = np.zeros(8, np.float32)
    for idx, b in zip(GRADES, inputs["o_bs"]):
        bo[idx] = np.asarray(b, np.float32)
    out = np.zeros_like(x)
    for b in range(B):
        for h in range(H):
            q = x[b] @ (Wq[h] * SCALE) + bq[h] * SCALE
            k = x[b] @ Wk[h] + bk[h]
            v = x[b] @ Wv[h] + bv[h]
            s = q @ k.T
            s = np.where(mask[b][None, :], -np.inf, s)
            s = s - s.max(axis=1, keepdims=True)
            p = np.exp(s)
            p /= p.sum(axis=1, keepdims=True)
            out[b] += (p @ v) @ Wo[h]
        out[b] += bo
    return out


def kernel(**inputs) -> np.ndarray:
    mask = np.asarray(inputs["key_padding_mask"])
    if mask.any():
        return _numpy_fallback(inputs)

    in_maps, bo = _host_inputs(inputs)
    if "nc" not in _NC_CACHE:
        _NC_CACHE["nc"] = build_nc()
    nc = _NC_CACHE["nc"]
    res = bass_utils.run_bass_kernel_spmd(nc, in_maps, core_ids=list(range(8)))
    out = np.zeros((B, T, MV), np.float32)
    for b in range(B):
        acc = res.results[2 * b]["out"] + res.results[2 * b + 1]["out"]
        out[b] = acc.T + bo
    return out
